# revision 18
# baseline (speedup 1.0000x reference)
"""AdaptiveSoftmax on 8 TRN2 NeuronCores.

Strategy: data-parallel over the 4096 rows (512 rows/core, no collectives).
Host-side prep (pure data movement / routing):
  - weight matrices transposed to [K, N] layout; fp8(e4m3, x64 scale) copies
    for the streamed log-sum-exp matmuls, bf16 for everything accuracy-
    critical (projections, target-logit gathers)
  - rows routed per-core so cluster-1 rows occupy the first T1B*128 "slots"
    and cluster-2 rows the last T2B*128 slots (adaptive-softmax dispatch);
    shortlist rows fill the gaps with masks = 0
  - gathered weight rows (W[target_index]) passed per-core so the target
    logit is a device-side dot product
Device side per core:
  - h1T/h2T tail projections (bf16 matmul)
  - streamed head/tail logit matmuls (fp8 DoubleRow for head+tail1, packed
    64-row tile_position pair for tail2) into fp32 PSUM, with ScalarE
    exp+row-sum fused epilogue; exp's scale=1/64 undoes the fp8 weight scale
    (fixed-shift log-sum-exp: logits here are O(5), exp never overflows)
  - target-logit dots via bf16 cross-matmul + identity-mask diag reduce
  - out = t_head - lse_head + m1*(t1 - lse1) + m2*(t2 - lse2); per-core
    loss partial sum on device
"""

import math
import os

import ml_dtypes
import numpy as np

VOCAB = 50257
D = 1024
C0, C1, C2 = 10000, 30000, 50257
SHORT = C0
HEAD_OUT = SHORT + 2  # 10002
T1_SIZE = C1 - C0  # 20000
T2_SIZE = C2 - C1  # 20257
D1, D2 = 256, 64
NCORES = 8
KT = D // 128  # 8 bf16 k-tiles
KT2 = D // 256  # 4 fp8 DoubleRow k-tiles
BF16 = ml_dtypes.bfloat16
FP8 = ml_dtypes.float8_e4m3
W_SCALE = 64.0

SC_W = 2048  # superchunk width (4 PSUM banks)
MM_N = 512  # matmul free dim / PSUM bank width


def _superchunks(total, width=SC_W):
    out = []
    v = 0
    while v < total:
        w = min(width, total - v)
        out.append((v, w))
        v += w
    return out


def _chunks(w, width=MM_N):
    out = []
    c = 0
    while c < w:
        out.append((c, min(width, w - c)))
        c += width
    return out


def _fp8(a):
    return np.clip(a, -240.0, 240.0).astype(FP8)


_NC_CACHE = {}


def _build_nc(T1B, T2B):
    """Build + compile the SPMD program for slot-block counts (T1B, T2B)."""
    key = (T1B, T2B)
    if key in _NC_CACHE:
        return _NC_CACHE[key]

    import concourse.bass as bass
    import concourse.tile as tile
    from concourse import bacc, mybir
    from concourse.masks import make_identity

    G = T1B + T2B
    R = G * 128  # slots per core
    R1 = T1B * 128
    R2 = T2B * 128
    f32 = mybir.dt.float32
    bf16 = mybir.dt.bfloat16
    fp8 = mybir.dt.float8e4
    AF = mybir.ActivationFunctionType
    DR = mybir.MatmulPerfMode.DoubleRow
    X = mybir.AxisListType.X

    nc = bacc.Bacc("TRN2", target_bir_lowering=False, debug=False)

    xT_d = nc.declare_dram_parameter("xT", [D, R], bf16, isOutput=False)
    xT8_d = nc.declare_dram_parameter("xT8", [D, R], fp8, isOutput=False)
    WgT_d = nc.declare_dram_parameter("WgT", [D, R], bf16, isOutput=False)
    Wg1T_d = nc.declare_dram_parameter("Wg1T", [D1, R1], bf16, isOutput=False)
    Wg2T_d = nc.declare_dram_parameter("Wg2T", [D2, R2], bf16, isOutput=False)
    WhT8_d = nc.declare_dram_parameter("WhT8", [D, HEAD_OUT], fp8, isOutput=False)
    Wp1T_d = nc.declare_dram_parameter("Wp1T", [D, D1], bf16, isOutput=False)
    Wo1T8_d = nc.declare_dram_parameter("Wo1T8", [D1, T1_SIZE], fp8, isOutput=False)
    Wp2Td_d = nc.declare_dram_parameter("Wp2Td", [D, 128], bf16, isOutput=False)
    Wo2T_d = nc.declare_dram_parameter("Wo2T", [D2, T2_SIZE], bf16, isOutput=False)
    m1_d = nc.declare_dram_parameter("m1", [128, T1B], f32, isOutput=False)
    m2_d = nc.declare_dram_parameter("m2", [128, T2B], f32, isOutput=False)
    mv_d = nc.declare_dram_parameter("mv", [128, G], f32, isOutput=False)
    og_d = nc.declare_dram_parameter("out_grid", [128, G], f32, isOutput=True)
    loss_d = nc.declare_dram_parameter("loss_part", [1, 1], f32, isOutput=True)

    head_plan = _superchunks(HEAD_OUT)
    t1_plan = _superchunks(T1_SIZE)
    t2_plan = _superchunks(T2_SIZE)
    inv_s = 1.0 / W_SCALE

    with tile.TileContext(nc) as tc:
        with (
            tc.tile_pool(name="const", bufs=1) as const,
            tc.tile_pool(name="slab", bufs=3) as slabp,
            tc.tile_pool(name="scr", bufs=2) as scrp,
            tc.tile_pool(name="mainps", bufs=2, space="PSUM") as mainps,
        ):
            # ---- critical-path inputs first (qSP ring order = program order) ----
            xT8 = const.tile([128, KT2, 2, R], fp8)

            def xT8_dma(kt):
                nc.sync.dma_start(
                    out=xT8[:, kt, :, :],
                    in_=xT8_d[kt * 256 : (kt + 1) * 256, :].rearrange(
                        "(i p) r -> p i r", p=128
                    ),
                )

            xT8_dma(0)

            ssum_h = const.tile([128, G, len(head_plan)], f32)
            ssum_1 = const.tile([128, T1B, len(t1_plan)], f32)
            ssum_2 = const.tile([128, T2B, len(t2_plan)], f32)

            def head_slab(isc):
                v0, w = head_plan[isc]
                slabH = slabp.tile([128, KT2, 2, SC_W], fp8, tag="slabH", name=f"slabH{isc}")
                for kt in range(KT2):
                    nc.sync.dma_start(
                        out=slabH[:, kt, :, :w],
                        in_=WhT8_d[kt * 256 : (kt + 1) * 256, v0 : v0 + w].rearrange(
                            "(i p) v -> p i v", p=128
                        ),
                    )
                return slabH

            def head_tile(slabH, isc, rb):
                v0, w = head_plan[isc]
                ps = mainps.tile([128, SC_W], f32, tag="ps", name=f"psh{isc}_{rb}")
                for kt in range(KT2):
                    for c0, cw in _chunks(w):
                        nc.tensor.matmul(
                            ps[:, c0 : c0 + cw],
                            xT8[:, kt, :, rb * 128 : (rb + 1) * 128],
                            slabH[:, kt, :, c0 : c0 + cw],
                            start=(kt == 0),
                            stop=(kt == KT2 - 1),
                            perf_mode=DR,
                        )
                scr = scrp.tile([128, SC_W], bf16, tag="scr", name=f"scrh{isc}_{rb}")
                nc.scalar.activation(
                    out=scr[:, :w],
                    in_=ps[:, :w],
                    func=AF.Exp,
                    scale=inv_s,
                    accum_out=ssum_h[:, rb, isc : isc + 1],
                )

            def t1_slab(isc):
                v0, w = t1_plan[isc]
                slab1 = slabp.tile([128, 2, SC_W], fp8, tag="slab1", name=f"slab1_{isc}")
                nc.sync.dma_start(
                    out=slab1[:, :, :w],
                    in_=Wo1T8_d[:, v0 : v0 + w].rearrange("(i p) v -> p i v", p=128),
                )
                return slab1

            def t1_tile(slab1, isc, rb):
                v0, w = t1_plan[isc]
                ps = mainps.tile([128, SC_W], f32, tag="ps", name=f"ps1_{isc}_{rb}")
                for c0, cw in _chunks(w):
                    nc.tensor.matmul(
                        ps[:, c0 : c0 + cw],
                        h1T8[:, :, rb * 128 : (rb + 1) * 128],
                        slab1[:, :, c0 : c0 + cw],
                        start=True,
                        stop=True,
                        perf_mode=DR,
                    )
                scr = scrp.tile([128, SC_W], bf16, tag="scr", name=f"scr1_{isc}_{rb}")
                nc.scalar.activation(
                    out=scr[:, :w],
                    in_=ps[:, :w],
                    func=AF.Exp,
                    scale=inv_s,
                    accum_out=ssum_1[:, rb, isc : isc + 1],
                )

            def t2_sc(isc):
                v0, w = t2_plan[isc]
                slab2 = slabp.tile([128, SC_W], bf16, tag="slab2", name=f"slab2_{isc}")
                nc.sync.dma_start(out=slab2[0:64, :w], in_=Wo2T_d[:, v0 : v0 + w])
                nc.sync.dma_start(out=slab2[64:128, :w], in_=Wo2T_d[:, v0 : v0 + w])
                psA = mainps.tile([128, SC_W], f32, tag="ps", name=f"ps2a_{isc}")
                psB = mainps.tile([128, SC_W], f32, tag="ps", name=f"ps2b_{isc}")
                for c0, cw in _chunks(w):
                    nc.tensor.matmul(
                        psA[:, c0 : c0 + cw],
                        h2T2[0:64, :],
                        slab2[0:64, c0 : c0 + cw],
                        start=True,
                        stop=True,
                        tile_position=(0, 0),
                    )
                    nc.tensor.matmul(
                        psB[:, c0 : c0 + cw],
                        h2T2[64:128, :],
                        slab2[64:128, c0 : c0 + cw],
                        start=True,
                        stop=True,
                        tile_position=(64, 0),
                    )
                for rb, pst in ((0, psA), (1, psB)):
                    scr = scrp.tile([128, SC_W], bf16, tag="scr", name=f"scr2_{isc}_{rb}")
                    nc.scalar.activation(
                        out=scr[:, :w],
                        in_=pst[:, :w],
                        func=AF.Exp,
                        accum_out=ssum_2[:, rb, isc : isc + 1],
                    )

            # head sc0 first: PE's first work only needs xT8[kt0] + slabH0[kt0]
            sH = head_slab(0)
            for kt in range(1, KT2):
                xT8_dma(kt)
            for rb in range(G):
                head_tile(sH, 0, rb)

            # remaining inputs stream while sc0 computes
            xT = const.tile([128, KT, R], bf16)
            nc.sync.dma_start(out=xT, in_=xT_d[:, :].rearrange("(kt p) r -> p kt r", p=128))
            Wp1T = const.tile([128, KT, D1], bf16)
            nc.sync.dma_start(out=Wp1T, in_=Wp1T_d[:, :].rearrange("(kt p) r -> p kt r", p=128))
            Wp2Td = const.tile([128, KT, 128], bf16)
            nc.sync.dma_start(out=Wp2Td, in_=Wp2Td_d[:, :].rearrange("(kt p) r -> p kt r", p=128))
            # ---- stage A: tail projections ----
            h1T = const.tile([128, 2, R1], bf16)
            h1T8 = const.tile([128, 2, R1], fp8)
            for m in range(2):
                ph = mainps.tile([128, SC_W], f32, tag="ps", name=f"ph{m}")
                for k in range(KT):
                    nc.tensor.matmul(
                        ph[:, :R1],
                        Wp1T[:, k, m * 128 : (m + 1) * 128],
                        xT[:, k, 0:R1],
                        start=(k == 0),
                        stop=(k == KT - 1),
                    )
                nc.vector.tensor_copy(h1T[:, m, :], ph[:, :R1])
                nc.vector.tensor_copy(h1T8[:, m, :], ph[:, :R1])
            h2T = const.tile([64, R2], bf16)
            h2T2 = const.tile([128, 128], bf16)
            ph2 = mainps.tile([128, SC_W], f32, tag="ps", name="ph2")
            for k in range(KT):
                nc.tensor.matmul(
                    ph2[:, :R2],
                    Wp2Td[:, k, :],
                    xT[:, k, R1:R],
                    start=(k == 0),
                    stop=(k == KT - 1),
                )
            nc.vector.tensor_copy(h2T[:, :], ph2[:64, :R2])
            nc.vector.tensor_copy(h2T2[0:64, :], ph2[0:64, 0:128])
            nc.vector.tensor_copy(h2T2[64:128, :], ph2[64:128, 128:256])

            # ---- head phase with t1 infill (ACT slack ~1.6us per head tile) ----
            t1_next = 0
            slab1_cur = None

            def emit_t1(k):
                nonlocal t1_next, slab1_cur
                for _ in range(k):
                    if t1_next >= len(t1_plan) * T1B:
                        return
                    isc, rb = divmod(t1_next, T1B)
                    if rb == 0:
                        slab1_cur = t1_slab(isc)
                    t1_tile(slab1_cur, isc, rb)
                    t1_next += 1

            for isc in range(1, len(head_plan)):
                sH = head_slab(isc)
                for rb in range(G):
                    head_tile(sH, isc, rb)
                    if rb % 2 == 1:
                        emit_t1(1)

            # ---- tail phase: rest of t1 + all t2 ----
            for isc in range(len(t2_plan)):
                emit_t1(2)
                t2_sc(isc)
            emit_t1(10**9)

            WgT = const.tile([128, KT, R], bf16)
            nc.scalar.dma_start(out=WgT, in_=WgT_d[:, :].rearrange("(kt p) r -> p kt r", p=128))
            Wg1T = const.tile([128, 2, R1], bf16)
            nc.scalar.dma_start(out=Wg1T, in_=Wg1T_d[:, :].rearrange("(kt p) r -> p kt r", p=128))
            Wg2T = const.tile([64, R2], bf16)
            nc.scalar.dma_start(out=Wg2T, in_=Wg2T_d[:, :])
            m1 = const.tile([128, T1B], f32)
            nc.scalar.dma_start(out=m1, in_=m1_d[:, :])
            m2 = const.tile([128, T2B], f32)
            nc.scalar.dma_start(out=m2, in_=m2_d[:, :])
            mv = const.tile([128, G], f32)
            nc.scalar.dma_start(out=mv, in_=mv_d[:, :])
            ident = const.tile([128, 128], f32)
            make_identity(nc, ident[:, :])
            ones = const.tile([128, 1], f32)
            nc.vector.memset(ones, 1.0)


            # ---- stage E: target-logit dots (cross-matmul + diag reduce) ----
            Th = const.tile([128, G], f32)
            T1 = const.tile([128, T1B], f32)
            T2 = const.tile([128, T2B], f32)
            dscr = const.tile([128, 128], f32)
            for rb in range(G):
                pd = mainps.tile([128, SC_W], f32, tag="ps")
                sl = slice(rb * 128, (rb + 1) * 128)
                for k in range(KT):
                    nc.tensor.matmul(
                        pd[:, :128], WgT[:, k, sl], xT[:, k, sl],
                        start=(k == 0), stop=(k == KT - 1),
                    )
                nc.vector.tensor_mul(dscr[:, :], pd[:, :128], ident[:, :])
                nc.vector.reduce_sum(Th[:, rb : rb + 1], dscr[:, :], axis=X)
            for rb in range(T1B):
                pd = mainps.tile([128, SC_W], f32, tag="ps")
                sl = slice(rb * 128, (rb + 1) * 128)
                for k in range(2):
                    nc.tensor.matmul(
                        pd[:, :128], Wg1T[:, k, sl], h1T[:, k, sl],
                        start=(k == 0), stop=(k == 1),
                    )
                nc.vector.tensor_mul(dscr[:, :], pd[:, :128], ident[:, :])
                nc.vector.reduce_sum(T1[:, rb : rb + 1], dscr[:, :], axis=X)
            for rb in range(T2B):
                pd = mainps.tile([128, SC_W], f32, tag="ps")
                sl = slice(rb * 128, (rb + 1) * 128)
                nc.tensor.matmul(
                    pd[:, :128], Wg2T[:, sl], h2T[:, sl], start=True, stop=True
                )
                nc.vector.tensor_mul(dscr[:, :], pd[:, :128], ident[:, :])
                nc.vector.reduce_sum(T2[:, rb : rb + 1], dscr[:, :], axis=X)

            # ---- stage F: reduce sums, lse, assemble ----
            S_h = const.tile([128, G], f32)
            nc.vector.reduce_sum(S_h, ssum_h, axis=X)
            S_1 = const.tile([128, T1B], f32)
            nc.vector.reduce_sum(S_1, ssum_1, axis=X)
            S_2 = const.tile([128, T2B], f32)
            nc.vector.reduce_sum(S_2, ssum_2, axis=X)

            lse_h = const.tile([128, G], f32)
            nc.scalar.activation(out=lse_h, in_=S_h, func=AF.Ln)
            lse_1 = const.tile([128, T1B], f32)
            nc.scalar.activation(out=lse_1, in_=S_1, func=AF.Ln)
            lse_2 = const.tile([128, T2B], f32)
            nc.scalar.activation(out=lse_2, in_=S_2, func=AF.Ln)

            og = const.tile([128, G], f32)
            nc.vector.tensor_sub(og, Th, lse_h)
            v1 = const.tile([128, T1B], f32)
            nc.vector.tensor_sub(v1, T1, lse_1)
            nc.vector.tensor_mul(v1, v1, m1)
            nc.vector.tensor_add(og[:, 0:T1B], og[:, 0:T1B], v1)
            v2 = const.tile([128, T2B], f32)
            nc.vector.tensor_sub(v2, T2, lse_2)
            nc.vector.tensor_mul(v2, v2, m2)
            nc.vector.tensor_add(og[:, T1B:G], og[:, T1B:G], v2)

            lsrc = const.tile([128, G], f32)
            nc.vector.tensor_mul(lsrc, og, mv)
            pl = mainps.tile([128, SC_W], f32, tag="ps")
            nc.tensor.matmul(pl[:1, :G], ones[:, :], lsrc[:, :], start=True, stop=True)
            lsum = const.tile([1, 1], f32)
            nc.vector.reduce_sum(lsum, pl[:1, :G], axis=X)

            nc.sync.dma_start(out=og_d[:, :], in_=og)
            nc.sync.dma_start(out=loss_d[:, :], in_=lsum)

    nc.compile()
    _NC_CACHE[key] = nc
    return nc


def kernel(x, target, W_head, Wp1, Wo1, Wp2, Wo2):
    from concourse.bass_utils import run_bass_kernel_spmd

    x = np.asarray(x, dtype=np.float32)
    t = np.asarray(target).astype(np.int64)
    W_head = np.asarray(W_head, dtype=np.float32)
    Wp1 = np.asarray(Wp1, dtype=np.float32)
    Wo1 = np.asarray(Wo1, dtype=np.float32)
    Wp2 = np.asarray(Wp2, dtype=np.float32)
    Wo2 = np.asarray(Wo2, dtype=np.float32)
    n = x.shape[0]

    # ---- host routing (adaptive-softmax dispatch) ----
    gather_inds = np.where(t < C0, t, np.where(t < C1, SHORT, SHORT + 1))
    rel1 = np.clip(t - C0, 0, T1_SIZE - 1)
    rel2 = np.clip(t - C1, 0, T2_SIZE - 1)
    in1 = (t >= C0) & (t < C1)
    in2 = (t >= C1) & (t < C2)
    idx1 = np.nonzero(in1)[0]
    idx2 = np.nonzero(in2)[0]
    idx0 = np.nonzero(~(in1 | in2))[0]

    per1 = [list(idx1[c::NCORES]) for c in range(NCORES)]
    per2 = [list(idx2[c::NCORES]) for c in range(NCORES)]
    fill = list(idx0)
    rpc = n // NCORES
    n1max = max(len(l) for l in per1)
    n2max = max(len(l) for l in per2)
    T1B = max(1, math.ceil(n1max / 128))
    T2B = max(1, math.ceil(n2max / 128))
    while (T1B + T2B) * 128 < rpc:
        if T1B <= T2B:
            T1B += 1
        else:
            T2B += 1
    G = T1B + T2B
    R, R1, R2 = G * 128, T1B * 128, T2B * 128

    # assign filler (shortlist) rows: each core needs rpc - n1c - n2c of them
    slot_rows = []  # per core: array of orig row index per slot, -1 = pad
    fpos = 0
    for c in range(NCORES):
        need = rpc - len(per1[c]) - len(per2[c])
        fillers = fill[fpos : fpos + need]
        fpos += need
        gap1 = R1 - len(per1[c])  # tail1-region filler slots
        f1 = fillers[:gap1]
        f2 = fillers[gap1:]
        rows = np.full(R, -1, dtype=np.int64)
        rows[: len(per1[c])] = per1[c]
        rows[len(per1[c]) : len(per1[c]) + len(f1)] = f1
        rows[R1 : R1 + len(per2[c])] = per2[c]
        rows[R1 + len(per2[c]) : R1 + len(per2[c]) + len(f2)] = f2
        slot_rows.append(rows)
    assert fpos == len(fill)

    # ---- shared (replicated) weight arrays ----
    WhT8 = _fp8(np.ascontiguousarray(W_head.T) * W_SCALE)
    Wp1T = np.ascontiguousarray(Wp1.T).astype(BF16)
    Wo1T8 = _fp8(np.ascontiguousarray(Wo1.T) * W_SCALE)
    Wp2Td = np.ascontiguousarray(np.concatenate([Wp2.T, Wp2.T], axis=1)).astype(BF16)
    Wo2T = np.ascontiguousarray(Wo2.T).astype(BF16)

    in_maps = []
    for c in range(NCORES):
        rows = slot_rows[c]
        valid = rows >= 0
        rv = np.where(valid, rows, 0)

        xT_f = np.where(valid[None, :], x[rv].T, 0.0)
        xT_c = xT_f.astype(BF16)
        xT8_c = _fp8(xT_f)
        WgT_c = np.where(valid[None, :], W_head[gather_inds[rv]].T, 0.0).astype(BF16)
        r1 = rows[:R1]
        v1 = r1 >= 0
        rv1 = np.where(v1, r1, 0)
        Wg1T_c = np.where(v1[None, :], Wo1[rel1[rv1]].T, 0.0).astype(BF16)
        r2 = rows[R1:]
        v2 = r2 >= 0
        rv2 = np.where(v2, r2, 0)
        Wg2T_c = np.where(v2[None, :], Wo2[rel2[rv2]].T, 0.0).astype(BF16)

        def grid(vec):
            return np.ascontiguousarray(vec.reshape(-1, 128).T).astype(np.float32)

        m1_c = grid((v1 & in1[rv1]).astype(np.float32))
        m2_c = grid((v2 & in2[rv2]).astype(np.float32))
        mv_c = grid(valid.astype(np.float32))

        in_maps.append(
            {
                "xT": np.ascontiguousarray(xT_c),
                "xT8": np.ascontiguousarray(xT8_c),
                "WgT": np.ascontiguousarray(WgT_c),
                "Wg1T": np.ascontiguousarray(Wg1T_c),
                "Wg2T": np.ascontiguousarray(Wg2T_c),
                "WhT8": WhT8,
                "Wp1T": Wp1T,
                "Wo1T8": Wo1T8,
                "Wp2Td": Wp2Td,
                "Wo2T": Wo2T,
                "m1": m1_c,
                "m2": m2_c,
                "mv": mv_c,
            }
        )

    nc = _build_nc(T1B, T2B)
    res = run_bass_kernel_spmd(
        nc,
        in_maps,
        core_ids=list(range(NCORES)),
        trace=bool(os.environ.get("AXS_TRACE")),
    )
    global LAST_RESULT
    LAST_RESULT = res

    out = np.zeros(n, dtype=np.float32)
    loss_sum = 0.0
    for c in range(NCORES):
        rows = slot_rows[c]
        valid = rows >= 0
        flat = np.asarray(res.results[c]["out_grid"]).T.reshape(-1)
        out[rows[valid]] = flat[valid]
        loss_sum += float(np.asarray(res.results[c]["loss_part"]).reshape(-1)[0])
    loss = np.float32(-loss_sum / n)
    return out, loss


LAST_RESULT = None


# revision 19
# speedup vs baseline: 1.1202x; 1.1202x over previous
"""AdaptiveSoftmax on 8 TRN2 NeuronCores.

Strategy: data-parallel over the 4096 rows (512 rows/core, no collectives).
Host-side prep (pure data movement / routing):
  - weight matrices transposed to [K, N] layout; fp8(e4m3, x64 scale) copies
    for the streamed log-sum-exp matmuls, bf16 for everything accuracy-
    critical (projections, target-logit gathers)
  - rows routed per-core so cluster-1 rows occupy the first T1B*128 "slots"
    and cluster-2 rows the last T2B*128 slots (adaptive-softmax dispatch);
    shortlist rows fill the gaps with masks = 0
  - gathered weight rows (W[target_index]) passed per-core so the target
    logit is a device-side dot product
Device side per core:
  - h1T/h2T tail projections (bf16 matmul)
  - streamed head/tail logit matmuls (fp8 DoubleRow for head+tail1, packed
    64-row tile_position pair for tail2) into fp32 PSUM, with ScalarE
    exp+row-sum fused epilogue; exp's scale=1/64 undoes the fp8 weight scale
    (fixed-shift log-sum-exp: logits here are O(5), exp never overflows)
  - target-logit dots via bf16 cross-matmul + identity-mask diag reduce
  - out = t_head - lse_head + m1*(t1 - lse1) + m2*(t2 - lse2); per-core
    loss partial sum on device
"""

import math
import os

import ml_dtypes
import numpy as np

VOCAB = 50257
D = 1024
C0, C1, C2 = 10000, 30000, 50257
SHORT = C0
HEAD_OUT = SHORT + 2  # 10002
T1_SIZE = C1 - C0  # 20000
T2_SIZE = C2 - C1  # 20257
D1, D2 = 256, 64
NCORES = 8
KT = D // 128  # 8 bf16 k-tiles
KT2 = D // 256  # 4 fp8 DoubleRow k-tiles
BF16 = ml_dtypes.bfloat16
FP8 = ml_dtypes.float8_e4m3
W_SCALE = 64.0

SC_W = 2048  # superchunk width (4 PSUM banks)
MM_N = 512  # matmul free dim / PSUM bank width


def _superchunks(total, width=SC_W):
    out = []
    v = 0
    while v < total:
        w = min(width, total - v)
        out.append((v, w))
        v += w
    return out


def _chunks(w, width=MM_N):
    out = []
    c = 0
    while c < w:
        out.append((c, min(width, w - c)))
        c += width
    return out


def _fp8(a):
    return np.clip(a, -240.0, 240.0).astype(FP8)


_NC_CACHE = {}


def _build_nc(T1B, T2B):
    """Build + compile the SPMD program for slot-block counts (T1B, T2B)."""
    key = (T1B, T2B)
    if key in _NC_CACHE:
        return _NC_CACHE[key]

    import concourse.bass as bass
    import concourse.tile as tile
    from concourse import bacc, mybir
    from concourse.masks import make_identity

    G = T1B + T2B
    R = G * 128  # slots per core
    R1 = T1B * 128
    R2 = T2B * 128
    f32 = mybir.dt.float32
    bf16 = mybir.dt.bfloat16
    fp8 = mybir.dt.float8e4
    AF = mybir.ActivationFunctionType
    DR = mybir.MatmulPerfMode.DoubleRow
    X = mybir.AxisListType.X

    nc = bacc.Bacc("TRN2", target_bir_lowering=False, debug=False)

    xT_d = nc.declare_dram_parameter("xT", [D, R], bf16, isOutput=False)
    xT8_d = nc.declare_dram_parameter("xT8", [D, R], fp8, isOutput=False)
    WgT_d = nc.declare_dram_parameter("WgT", [D, R], bf16, isOutput=False)
    Wg1T_d = nc.declare_dram_parameter("Wg1T", [D1, R1], bf16, isOutput=False)
    Wg2T_d = nc.declare_dram_parameter("Wg2T", [D2, R2], bf16, isOutput=False)
    WhT8_d = nc.declare_dram_parameter("WhT8", [D, HEAD_OUT], fp8, isOutput=False)
    Wp1T_d = nc.declare_dram_parameter("Wp1T", [D, D1], bf16, isOutput=False)
    Wo1T8_d = nc.declare_dram_parameter("Wo1T8", [D1, T1_SIZE], fp8, isOutput=False)
    Wp2Td_d = nc.declare_dram_parameter("Wp2Td", [D, 128], bf16, isOutput=False)
    Wo2T_d = nc.declare_dram_parameter("Wo2T", [D2, T2_SIZE], bf16, isOutput=False)
    m1_d = nc.declare_dram_parameter("m1", [128, T1B], f32, isOutput=False)
    m2_d = nc.declare_dram_parameter("m2", [128, T2B], f32, isOutput=False)
    mv_d = nc.declare_dram_parameter("mv", [128, G], f32, isOutput=False)
    og_d = nc.declare_dram_parameter("out_grid", [128, G], f32, isOutput=True)
    loss_d = nc.declare_dram_parameter("loss_part", [1, 1], f32, isOutput=True)

    head_plan = _superchunks(HEAD_OUT)
    t1_plan = _superchunks(T1_SIZE)
    t2_plan = _superchunks(T2_SIZE)
    inv_s = 1.0 / W_SCALE

    with tile.TileContext(nc) as tc:
        with (
            tc.tile_pool(name="const", bufs=1) as const,
            tc.tile_pool(name="slab", bufs=3) as slabp,
            tc.tile_pool(name="scr", bufs=2) as scrp,
            tc.tile_pool(name="mainps", bufs=2, space="PSUM") as mainps,
        ):
            # ---- critical-path inputs first (qSP ring order = program order) ----
            xT8 = const.tile([128, KT2, 2, R], fp8)

            def xT8_dma(kt):
                nc.sync.dma_start(
                    out=xT8[:, kt, :, :],
                    in_=xT8_d[kt * 256 : (kt + 1) * 256, :].rearrange(
                        "(i p) r -> p i r", p=128
                    ),
                )

            xT8_dma(0)

            ssum_h = const.tile([128, G, len(head_plan)], f32)
            ssum_1 = const.tile([128, T1B, len(t1_plan)], f32)
            ssum_2 = const.tile([128, T2B, len(t2_plan)], f32)

            def head_slab(isc):
                v0, w = head_plan[isc]
                slabH = slabp.tile([128, KT2, 2, SC_W], fp8, tag="slabH", name=f"slabH{isc}")
                for kt in range(KT2):
                    nc.sync.dma_start(
                        out=slabH[:, kt, :, :w],
                        in_=WhT8_d[kt * 256 : (kt + 1) * 256, v0 : v0 + w].rearrange(
                            "(i p) v -> p i v", p=128
                        ),
                    )
                return slabH

            def head_tile(slabH, isc, rb):
                v0, w = head_plan[isc]
                ps = mainps.tile([128, SC_W], f32, tag="ps", name=f"psh{isc}_{rb}")
                for kt in range(KT2):
                    for c0, cw in _chunks(w):
                        nc.tensor.matmul(
                            ps[:, c0 : c0 + cw],
                            xT8[:, kt, :, rb * 128 : (rb + 1) * 128],
                            slabH[:, kt, :, c0 : c0 + cw],
                            start=(kt == 0),
                            stop=(kt == KT2 - 1),
                            perf_mode=DR,
                        )
                scr = scrp.tile([128, SC_W], bf16, tag="scr", name=f"scrh{isc}_{rb}")
                nc.scalar.activation(
                    out=scr[:, :w],
                    in_=ps[:, :w],
                    func=AF.Exp,
                    scale=inv_s,
                    accum_out=ssum_h[:, rb, isc : isc + 1],
                )

            def t1_slab(isc):
                v0, w = t1_plan[isc]
                slab1 = slabp.tile([128, 2, SC_W], fp8, tag="slab1", name=f"slab1_{isc}")
                nc.sync.dma_start(
                    out=slab1[:, :, :w],
                    in_=Wo1T8_d[:, v0 : v0 + w].rearrange("(i p) v -> p i v", p=128),
                )
                return slab1

            def t1_tile(slab1, isc, rb):
                v0, w = t1_plan[isc]
                ps = mainps.tile([128, SC_W], f32, tag="ps", name=f"ps1_{isc}_{rb}")
                for c0, cw in _chunks(w):
                    nc.tensor.matmul(
                        ps[:, c0 : c0 + cw],
                        h1T8[:, :, rb * 128 : (rb + 1) * 128],
                        slab1[:, :, c0 : c0 + cw],
                        start=True,
                        stop=True,
                        perf_mode=DR,
                    )
                scr = scrp.tile([128, SC_W], bf16, tag="scr", name=f"scr1_{isc}_{rb}")
                nc.scalar.activation(
                    out=scr[:, :w],
                    in_=ps[:, :w],
                    func=AF.Exp,
                    scale=inv_s,
                    accum_out=ssum_1[:, rb, isc : isc + 1],
                )

            def t2_sc(isc):
                v0, w = t2_plan[isc]
                slab2 = slabp.tile([128, SC_W], bf16, tag="slab2", name=f"slab2_{isc}")
                nc.sync.dma_start(out=slab2[0:64, :w], in_=Wo2T_d[:, v0 : v0 + w])
                nc.sync.dma_start(out=slab2[64:128, :w], in_=Wo2T_d[:, v0 : v0 + w])
                psA = mainps.tile([128, SC_W], f32, tag="ps", name=f"ps2a_{isc}")
                psB = mainps.tile([128, SC_W], f32, tag="ps", name=f"ps2b_{isc}")
                for c0, cw in _chunks(w):
                    nc.tensor.matmul(
                        psA[:, c0 : c0 + cw],
                        h2T2[0:64, :],
                        slab2[0:64, c0 : c0 + cw],
                        start=True,
                        stop=True,
                        tile_position=(0, 0),
                    )
                    nc.tensor.matmul(
                        psB[:, c0 : c0 + cw],
                        h2T2[64:128, :],
                        slab2[64:128, c0 : c0 + cw],
                        start=True,
                        stop=True,
                        tile_position=(64, 0),
                    )
                for rb, pst in ((0, psA), (1, psB)):
                    scr = scrp.tile([128, SC_W], bf16, tag="scr", name=f"scr2_{isc}_{rb}")
                    nc.scalar.activation(
                        out=scr[:, :w],
                        in_=pst[:, :w],
                        func=AF.Exp,
                        accum_out=ssum_2[:, rb, isc : isc + 1],
                    )

            # head sc0 first: PE's first work only needs xT8[kt0] + slabH0[kt0]
            sH = head_slab(0)
            for kt in range(1, KT2):
                xT8_dma(kt)
            for rb in range(G):
                head_tile(sH, 0, rb)

            # remaining inputs stream while sc0 computes
            xT = const.tile([128, KT, R], bf16)
            nc.sync.dma_start(out=xT, in_=xT_d[:, :].rearrange("(kt p) r -> p kt r", p=128))
            Wp1T = const.tile([128, KT, D1], bf16)
            nc.sync.dma_start(out=Wp1T, in_=Wp1T_d[:, :].rearrange("(kt p) r -> p kt r", p=128))
            Wp2Td = const.tile([128, KT, 128], bf16)
            nc.sync.dma_start(out=Wp2Td, in_=Wp2Td_d[:, :].rearrange("(kt p) r -> p kt r", p=128))
            # ---- stage A: tail projections ----
            h1T = const.tile([128, 2, R1], bf16)
            h1T8 = const.tile([128, 2, R1], fp8)
            for m in range(2):
                ph = mainps.tile([128, SC_W], f32, tag="ps", name=f"ph{m}")
                for k in range(KT):
                    nc.tensor.matmul(
                        ph[:, :R1],
                        Wp1T[:, k, m * 128 : (m + 1) * 128],
                        xT[:, k, 0:R1],
                        start=(k == 0),
                        stop=(k == KT - 1),
                    )
                nc.vector.tensor_copy(h1T[:, m, :], ph[:, :R1])
                nc.vector.tensor_copy(h1T8[:, m, :], ph[:, :R1])
            h2T = const.tile([64, R2], bf16)
            h2T2 = const.tile([128, 128], bf16)
            ph2 = mainps.tile([128, SC_W], f32, tag="ps", name="ph2")
            for k in range(KT):
                nc.tensor.matmul(
                    ph2[:, :R2],
                    Wp2Td[:, k, :],
                    xT[:, k, R1:R],
                    start=(k == 0),
                    stop=(k == KT - 1),
                )
            nc.vector.tensor_copy(h2T[:, :], ph2[:64, :R2])
            nc.vector.tensor_copy(h2T2[0:64, :], ph2[0:64, 0:128])
            nc.vector.tensor_copy(h2T2[64:128, :], ph2[64:128, 128:256])

            # ---- head phase with t1 infill (ACT slack ~1.6us per head tile) ----
            t1_next = 0
            slab1_cur = None

            def emit_t1(k):
                nonlocal t1_next, slab1_cur
                for _ in range(k):
                    if t1_next >= len(t1_plan) * T1B:
                        return
                    isc, rb = divmod(t1_next, T1B)
                    if rb == 0:
                        slab1_cur = t1_slab(isc)
                    t1_tile(slab1_cur, isc, rb)
                    t1_next += 1

            t2_next = 0

            def emit_t2(k):
                nonlocal t2_next
                for _ in range(k):
                    if t2_next >= len(t2_plan):
                        return
                    t2_sc(t2_next)
                    t2_next += 1

            for isc in range(1, len(head_plan)):
                sH = head_slab(isc)
                for rb in range(G):
                    head_tile(sH, isc, rb)
                    if rb % 2 == 1:
                        emit_t1(1)
                emit_t2(1)

            # ---- tail phase: rest of t1 + t2 ----
            while t1_next < len(t1_plan) * T1B or t2_next < len(t2_plan):
                emit_t1(2)
                emit_t2(1)

            WgT = const.tile([128, KT, R], bf16)
            nc.scalar.dma_start(out=WgT, in_=WgT_d[:, :].rearrange("(kt p) r -> p kt r", p=128))
            Wg1T = const.tile([128, 2, R1], bf16)
            nc.scalar.dma_start(out=Wg1T, in_=Wg1T_d[:, :].rearrange("(kt p) r -> p kt r", p=128))
            Wg2T = const.tile([64, R2], bf16)
            nc.scalar.dma_start(out=Wg2T, in_=Wg2T_d[:, :])
            m1 = const.tile([128, T1B], f32)
            nc.scalar.dma_start(out=m1, in_=m1_d[:, :])
            m2 = const.tile([128, T2B], f32)
            nc.scalar.dma_start(out=m2, in_=m2_d[:, :])
            mv = const.tile([128, G], f32)
            nc.scalar.dma_start(out=mv, in_=mv_d[:, :])
            ident = const.tile([128, 128], f32)
            make_identity(nc, ident[:, :])
            ones = const.tile([128, 1], f32)
            nc.vector.memset(ones, 1.0)


            # ---- stage E: target-logit dots (cross-matmul + diag reduce) ----
            Th = const.tile([128, G], f32)
            T1 = const.tile([128, T1B], f32)
            T2 = const.tile([128, T2B], f32)
            dscr = const.tile([128, 128], f32)
            for rb in range(G):
                pd = mainps.tile([128, SC_W], f32, tag="ps")
                sl = slice(rb * 128, (rb + 1) * 128)
                for k in range(KT):
                    nc.tensor.matmul(
                        pd[:, :128], WgT[:, k, sl], xT[:, k, sl],
                        start=(k == 0), stop=(k == KT - 1),
                    )
                nc.vector.tensor_mul(dscr[:, :], pd[:, :128], ident[:, :])
                nc.vector.reduce_sum(Th[:, rb : rb + 1], dscr[:, :], axis=X)
            for rb in range(T1B):
                pd = mainps.tile([128, SC_W], f32, tag="ps")
                sl = slice(rb * 128, (rb + 1) * 128)
                for k in range(2):
                    nc.tensor.matmul(
                        pd[:, :128], Wg1T[:, k, sl], h1T[:, k, sl],
                        start=(k == 0), stop=(k == 1),
                    )
                nc.vector.tensor_mul(dscr[:, :], pd[:, :128], ident[:, :])
                nc.vector.reduce_sum(T1[:, rb : rb + 1], dscr[:, :], axis=X)
            for rb in range(T2B):
                pd = mainps.tile([128, SC_W], f32, tag="ps")
                sl = slice(rb * 128, (rb + 1) * 128)
                nc.tensor.matmul(
                    pd[:, :128], Wg2T[:, sl], h2T[:, sl], start=True, stop=True
                )
                nc.vector.tensor_mul(dscr[:, :], pd[:, :128], ident[:, :])
                nc.vector.reduce_sum(T2[:, rb : rb + 1], dscr[:, :], axis=X)

            # ---- stage F: reduce sums, lse, assemble ----
            S_h = const.tile([128, G], f32)
            nc.vector.reduce_sum(S_h, ssum_h, axis=X)
            S_1 = const.tile([128, T1B], f32)
            nc.vector.reduce_sum(S_1, ssum_1, axis=X)
            S_2 = const.tile([128, T2B], f32)
            nc.vector.reduce_sum(S_2, ssum_2, axis=X)

            lse_h = const.tile([128, G], f32)
            nc.scalar.activation(out=lse_h, in_=S_h, func=AF.Ln)
            lse_1 = const.tile([128, T1B], f32)
            nc.scalar.activation(out=lse_1, in_=S_1, func=AF.Ln)
            lse_2 = const.tile([128, T2B], f32)
            nc.scalar.activation(out=lse_2, in_=S_2, func=AF.Ln)

            og = const.tile([128, G], f32)
            nc.vector.tensor_sub(og, Th, lse_h)
            v1 = const.tile([128, T1B], f32)
            nc.vector.tensor_sub(v1, T1, lse_1)
            nc.vector.tensor_mul(v1, v1, m1)
            nc.vector.tensor_add(og[:, 0:T1B], og[:, 0:T1B], v1)
            v2 = const.tile([128, T2B], f32)
            nc.vector.tensor_sub(v2, T2, lse_2)
            nc.vector.tensor_mul(v2, v2, m2)
            nc.vector.tensor_add(og[:, T1B:G], og[:, T1B:G], v2)

            lsrc = const.tile([128, G], f32)
            nc.vector.tensor_mul(lsrc, og, mv)
            pl = mainps.tile([128, SC_W], f32, tag="ps")
            nc.tensor.matmul(pl[:1, :G], ones[:, :], lsrc[:, :], start=True, stop=True)
            lsum = const.tile([1, 1], f32)
            nc.vector.reduce_sum(lsum, pl[:1, :G], axis=X)

            nc.sync.dma_start(out=og_d[:, :], in_=og)
            nc.sync.dma_start(out=loss_d[:, :], in_=lsum)

    nc.compile()
    _NC_CACHE[key] = nc
    return nc


def kernel(x, target, W_head, Wp1, Wo1, Wp2, Wo2):
    from concourse.bass_utils import run_bass_kernel_spmd

    x = np.asarray(x, dtype=np.float32)
    t = np.asarray(target).astype(np.int64)
    W_head = np.asarray(W_head, dtype=np.float32)
    Wp1 = np.asarray(Wp1, dtype=np.float32)
    Wo1 = np.asarray(Wo1, dtype=np.float32)
    Wp2 = np.asarray(Wp2, dtype=np.float32)
    Wo2 = np.asarray(Wo2, dtype=np.float32)
    n = x.shape[0]

    # ---- host routing (adaptive-softmax dispatch) ----
    gather_inds = np.where(t < C0, t, np.where(t < C1, SHORT, SHORT + 1))
    rel1 = np.clip(t - C0, 0, T1_SIZE - 1)
    rel2 = np.clip(t - C1, 0, T2_SIZE - 1)
    in1 = (t >= C0) & (t < C1)
    in2 = (t >= C1) & (t < C2)
    idx1 = np.nonzero(in1)[0]
    idx2 = np.nonzero(in2)[0]
    idx0 = np.nonzero(~(in1 | in2))[0]

    per1 = [list(idx1[c::NCORES]) for c in range(NCORES)]
    per2 = [list(idx2[c::NCORES]) for c in range(NCORES)]
    fill = list(idx0)
    rpc = n // NCORES
    n1max = max(len(l) for l in per1)
    n2max = max(len(l) for l in per2)
    T1B = max(1, math.ceil(n1max / 128))
    T2B = max(1, math.ceil(n2max / 128))
    while (T1B + T2B) * 128 < rpc:
        if T1B <= T2B:
            T1B += 1
        else:
            T2B += 1
    G = T1B + T2B
    R, R1, R2 = G * 128, T1B * 128, T2B * 128

    # assign filler (shortlist) rows: each core needs rpc - n1c - n2c of them
    slot_rows = []  # per core: array of orig row index per slot, -1 = pad
    fpos = 0
    for c in range(NCORES):
        need = rpc - len(per1[c]) - len(per2[c])
        fillers = fill[fpos : fpos + need]
        fpos += need
        gap1 = R1 - len(per1[c])  # tail1-region filler slots
        f1 = fillers[:gap1]
        f2 = fillers[gap1:]
        rows = np.full(R, -1, dtype=np.int64)
        rows[: len(per1[c])] = per1[c]
        rows[len(per1[c]) : len(per1[c]) + len(f1)] = f1
        rows[R1 : R1 + len(per2[c])] = per2[c]
        rows[R1 + len(per2[c]) : R1 + len(per2[c]) + len(f2)] = f2
        slot_rows.append(rows)
    assert fpos == len(fill)

    # ---- shared (replicated) weight arrays ----
    WhT8 = _fp8(np.ascontiguousarray(W_head.T) * W_SCALE)
    Wp1T = np.ascontiguousarray(Wp1.T).astype(BF16)
    Wo1T8 = _fp8(np.ascontiguousarray(Wo1.T) * W_SCALE)
    Wp2Td = np.ascontiguousarray(np.concatenate([Wp2.T, Wp2.T], axis=1)).astype(BF16)
    Wo2T = np.ascontiguousarray(Wo2.T).astype(BF16)

    in_maps = []
    for c in range(NCORES):
        rows = slot_rows[c]
        valid = rows >= 0
        rv = np.where(valid, rows, 0)

        xT_f = np.where(valid[None, :], x[rv].T, 0.0)
        xT_c = xT_f.astype(BF16)
        xT8_c = _fp8(xT_f)
        WgT_c = np.where(valid[None, :], W_head[gather_inds[rv]].T, 0.0).astype(BF16)
        r1 = rows[:R1]
        v1 = r1 >= 0
        rv1 = np.where(v1, r1, 0)
        Wg1T_c = np.where(v1[None, :], Wo1[rel1[rv1]].T, 0.0).astype(BF16)
        r2 = rows[R1:]
        v2 = r2 >= 0
        rv2 = np.where(v2, r2, 0)
        Wg2T_c = np.where(v2[None, :], Wo2[rel2[rv2]].T, 0.0).astype(BF16)

        def grid(vec):
            return np.ascontiguousarray(vec.reshape(-1, 128).T).astype(np.float32)

        m1_c = grid((v1 & in1[rv1]).astype(np.float32))
        m2_c = grid((v2 & in2[rv2]).astype(np.float32))
        mv_c = grid(valid.astype(np.float32))

        in_maps.append(
            {
                "xT": np.ascontiguousarray(xT_c),
                "xT8": np.ascontiguousarray(xT8_c),
                "WgT": np.ascontiguousarray(WgT_c),
                "Wg1T": np.ascontiguousarray(Wg1T_c),
                "Wg2T": np.ascontiguousarray(Wg2T_c),
                "WhT8": WhT8,
                "Wp1T": Wp1T,
                "Wo1T8": Wo1T8,
                "Wp2Td": Wp2Td,
                "Wo2T": Wo2T,
                "m1": m1_c,
                "m2": m2_c,
                "mv": mv_c,
            }
        )

    nc = _build_nc(T1B, T2B)
    res = run_bass_kernel_spmd(
        nc,
        in_maps,
        core_ids=list(range(NCORES)),
        trace=bool(os.environ.get("AXS_TRACE")),
    )
    global LAST_RESULT
    LAST_RESULT = res

    out = np.zeros(n, dtype=np.float32)
    loss_sum = 0.0
    for c in range(NCORES):
        rows = slot_rows[c]
        valid = rows >= 0
        flat = np.asarray(res.results[c]["out_grid"]).T.reshape(-1)
        out[rows[valid]] = flat[valid]
        loss_sum += float(np.asarray(res.results[c]["loss_part"]).reshape(-1)[0])
    loss = np.float32(-loss_sum / n)
    return out, loss


LAST_RESULT = None


# revision 20
# speedup vs baseline: 1.1763x; 1.0501x over previous
"""AdaptiveSoftmax on 8 TRN2 NeuronCores.

Strategy: data-parallel over the 4096 rows (512 rows/core, no collectives).
Host-side prep (pure data movement / routing):
  - weight matrices transposed to [K, N] layout; fp8(e4m3, x64 scale) copies
    for the streamed log-sum-exp matmuls, bf16 for everything accuracy-
    critical (projections, target-logit gathers)
  - rows routed per-core so cluster-1 rows occupy the first T1B*128 "slots"
    and cluster-2 rows the last T2B*128 slots (adaptive-softmax dispatch);
    shortlist rows fill the gaps with masks = 0
  - gathered weight rows (W[target_index]) passed per-core so the target
    logit is a device-side dot product
Device side per core:
  - h1T/h2T tail projections (bf16 matmul)
  - streamed head/tail logit matmuls (fp8 DoubleRow for head+tail1, packed
    64-row tile_position pair for tail2) into fp32 PSUM, with ScalarE
    exp+row-sum fused epilogue; exp's scale=1/64 undoes the fp8 weight scale
    (fixed-shift log-sum-exp: logits here are O(5), exp never overflows)
  - target-logit dots via bf16 cross-matmul + identity-mask diag reduce
  - out = t_head - lse_head + m1*(t1 - lse1) + m2*(t2 - lse2); per-core
    loss partial sum on device
"""

import math
import os

import ml_dtypes
import numpy as np

VOCAB = 50257
D = 1024
C0, C1, C2 = 10000, 30000, 50257
SHORT = C0
HEAD_OUT = SHORT + 2  # 10002
T1_SIZE = C1 - C0  # 20000
T2_SIZE = C2 - C1  # 20257
D1, D2 = 256, 64
NCORES = 8
KT = D // 128  # 8 bf16 k-tiles
KT2 = D // 256  # 4 fp8 DoubleRow k-tiles
BF16 = ml_dtypes.bfloat16
FP8 = ml_dtypes.float8_e4m3
W_SCALE = 64.0

SC_W = 2048  # superchunk width (4 PSUM banks)
MM_N = 512  # matmul free dim / PSUM bank width


def _superchunks(total, width=SC_W):
    out = []
    v = 0
    while v < total:
        w = min(width, total - v)
        out.append((v, w))
        v += w
    return out


def _chunks(w, width=MM_N):
    out = []
    c = 0
    while c < w:
        out.append((c, min(width, w - c)))
        c += width
    return out


def _fp8(a):
    return np.clip(a, -240.0, 240.0).astype(FP8)


_NC_CACHE = {}


def _build_nc(T1B, T2B):
    """Build + compile the SPMD program for slot-block counts (T1B, T2B)."""
    key = (T1B, T2B)
    if key in _NC_CACHE:
        return _NC_CACHE[key]

    import concourse.bass as bass
    import concourse.tile as tile
    from concourse import bacc, mybir
    from concourse.masks import make_identity

    G = T1B + T2B
    R = G * 128  # slots per core
    R1 = T1B * 128
    R2 = T2B * 128
    f32 = mybir.dt.float32
    bf16 = mybir.dt.bfloat16
    fp8 = mybir.dt.float8e4
    AF = mybir.ActivationFunctionType
    DR = mybir.MatmulPerfMode.DoubleRow
    X = mybir.AxisListType.X

    nc = bacc.Bacc("TRN2", target_bir_lowering=False, debug=False)

    xT_d = nc.declare_dram_parameter("xT", [D, R], bf16, isOutput=False)
    xT8_d = nc.declare_dram_parameter("xT8", [D, R], fp8, isOutput=False)
    WgT_d = nc.declare_dram_parameter("WgT", [D, R], bf16, isOutput=False)
    Wg1T_d = nc.declare_dram_parameter("Wg1T", [D1, R1], bf16, isOutput=False)
    Wg2T_d = nc.declare_dram_parameter("Wg2T", [D2, R2], bf16, isOutput=False)
    WhT8_d = nc.declare_dram_parameter("WhT8", [D, HEAD_OUT], fp8, isOutput=False)
    Wp1T_d = nc.declare_dram_parameter("Wp1T", [D, D1], bf16, isOutput=False)
    Wo1T8_d = nc.declare_dram_parameter("Wo1T8", [D1, T1_SIZE], fp8, isOutput=False)
    Wp2Td_d = nc.declare_dram_parameter("Wp2Td", [D, 128], bf16, isOutput=False)
    Wo2T_d = nc.declare_dram_parameter("Wo2T", [D2, T2_SIZE], bf16, isOutput=False)
    m1_d = nc.declare_dram_parameter("m1", [128, T1B], f32, isOutput=False)
    m2_d = nc.declare_dram_parameter("m2", [128, T2B], f32, isOutput=False)
    mv_d = nc.declare_dram_parameter("mv", [128, G], f32, isOutput=False)
    og_d = nc.declare_dram_parameter("out_grid", [128, G], f32, isOutput=True)
    loss_d = nc.declare_dram_parameter("loss_part", [1, 1], f32, isOutput=True)

    head_plan = _superchunks(HEAD_OUT)
    t1_plan = _superchunks(T1_SIZE)
    t2_plan = _superchunks(T2_SIZE)
    inv_s = 1.0 / W_SCALE

    with tile.TileContext(nc) as tc:
        with (
            tc.tile_pool(name="const", bufs=1) as const,
            tc.tile_pool(name="slab", bufs=3) as slabp,
            tc.tile_pool(name="scr", bufs=2) as scrp,
            tc.tile_pool(name="mainps", bufs=2, space="PSUM") as mainps,
        ):
            # ---- critical-path inputs first (qSP ring order = program order) ----
            xT8 = const.tile([128, KT2, 2, R], fp8)

            def xT8_dma(kt):
                nc.sync.dma_start(
                    out=xT8[:, kt, :, :],
                    in_=xT8_d[kt * 256 : (kt + 1) * 256, :].rearrange(
                        "(i p) r -> p i r", p=128
                    ),
                )

            xT8_dma(0)

            ssum_h = const.tile([128, G, len(head_plan)], f32)
            ssum_1 = const.tile([128, T1B, len(t1_plan)], f32)
            ssum_2 = const.tile([128, T2B, len(t2_plan)], f32)

            def head_slab(isc):
                v0, w = head_plan[isc]
                slabH = slabp.tile([128, KT2, 2, SC_W], fp8, tag="slabH", name=f"slabH{isc}")
                for kt in range(KT2):
                    nc.sync.dma_start(
                        out=slabH[:, kt, :, :w],
                        in_=WhT8_d[kt * 256 : (kt + 1) * 256, v0 : v0 + w].rearrange(
                            "(i p) v -> p i v", p=128
                        ),
                    )
                return slabH

            def head_tile(slabH, isc, rb):
                v0, w = head_plan[isc]
                ps = mainps.tile([128, SC_W], f32, tag="ps", name=f"psh{isc}_{rb}")
                for kt in range(KT2):
                    for c0, cw in _chunks(w):
                        nc.tensor.matmul(
                            ps[:, c0 : c0 + cw],
                            xT8[:, kt, :, rb * 128 : (rb + 1) * 128],
                            slabH[:, kt, :, c0 : c0 + cw],
                            start=(kt == 0),
                            stop=(kt == KT2 - 1),
                            perf_mode=DR,
                        )
                scr = scrp.tile([128, SC_W], bf16, tag="scr", name=f"scrh{isc}_{rb}")
                nc.scalar.activation(
                    out=scr[:, :w],
                    in_=ps[:, :w],
                    func=AF.Exp,
                    scale=inv_s,
                    accum_out=ssum_h[:, rb, isc : isc + 1],
                )

            def t1_slab(isc):
                v0, w = t1_plan[isc]
                slab1 = slabp.tile([128, 2, SC_W], fp8, tag="slab1", name=f"slab1_{isc}")
                nc.sync.dma_start(
                    out=slab1[:, :, :w],
                    in_=Wo1T8_d[:, v0 : v0 + w].rearrange("(i p) v -> p i v", p=128),
                )
                return slab1

            def t1_tile(slab1, isc, rb):
                v0, w = t1_plan[isc]
                ps = mainps.tile([128, SC_W], f32, tag="ps", name=f"ps1_{isc}_{rb}")
                for c0, cw in _chunks(w):
                    nc.tensor.matmul(
                        ps[:, c0 : c0 + cw],
                        h1T8[:, :, rb * 128 : (rb + 1) * 128],
                        slab1[:, :, c0 : c0 + cw],
                        start=True,
                        stop=True,
                        perf_mode=DR,
                    )
                scr = scrp.tile([128, SC_W], bf16, tag="scr", name=f"scr1_{isc}_{rb}")
                nc.scalar.activation(
                    out=scr[:, :w],
                    in_=ps[:, :w],
                    func=AF.Exp,
                    scale=inv_s,
                    accum_out=ssum_1[:, rb, isc : isc + 1],
                )

            def t2_sc(isc):
                v0, w = t2_plan[isc]
                slab2 = slabp.tile([128, SC_W], bf16, tag="slab2", name=f"slab2_{isc}")
                nc.sync.dma_start(out=slab2[0:64, :w], in_=Wo2T_d[:, v0 : v0 + w])
                nc.sync.dma_start(out=slab2[64:128, :w], in_=Wo2T_d[:, v0 : v0 + w])
                psA = mainps.tile([128, SC_W], f32, tag="ps", name=f"ps2a_{isc}")
                psB = mainps.tile([128, SC_W], f32, tag="ps", name=f"ps2b_{isc}")
                for c0, cw in _chunks(w):
                    nc.tensor.matmul(
                        psA[:, c0 : c0 + cw],
                        h2T2[0:64, :],
                        slab2[0:64, c0 : c0 + cw],
                        start=True,
                        stop=True,
                        tile_position=(0, 0),
                    )
                    nc.tensor.matmul(
                        psB[:, c0 : c0 + cw],
                        h2T2[64:128, :],
                        slab2[64:128, c0 : c0 + cw],
                        start=True,
                        stop=True,
                        tile_position=(64, 0),
                    )
                for rb, pst in ((0, psA), (1, psB)):
                    scr = scrp.tile([128, SC_W], bf16, tag="scr", name=f"scr2_{isc}_{rb}")
                    nc.scalar.activation(
                        out=scr[:, :w],
                        in_=pst[:, :w],
                        func=AF.Exp,
                        accum_out=ssum_2[:, rb, isc : isc + 1],
                    )

            # head sc0 first: PE's first work only needs xT8[kt0] + slabH0[kt0]
            sH = head_slab(0)
            for kt in range(1, KT2):
                xT8_dma(kt)
            for rb in range(G):
                head_tile(sH, 0, rb)

            # remaining inputs stream while sc0 computes
            xT = const.tile([128, KT, R], bf16)
            nc.sync.dma_start(out=xT, in_=xT_d[:, :].rearrange("(kt p) r -> p kt r", p=128))
            Wp1T = const.tile([128, KT, D1], bf16)
            nc.sync.dma_start(out=Wp1T, in_=Wp1T_d[:, :].rearrange("(kt p) r -> p kt r", p=128))
            Wp2Td = const.tile([128, KT, 128], bf16)
            nc.sync.dma_start(out=Wp2Td, in_=Wp2Td_d[:, :].rearrange("(kt p) r -> p kt r", p=128))
            # ---- stage A: tail projections ----
            h1T = const.tile([128, 2, R1], bf16)
            h1T8 = const.tile([128, 2, R1], fp8)
            for m in range(2):
                ph = mainps.tile([128, SC_W], f32, tag="ps", name=f"ph{m}")
                for k in range(KT):
                    nc.tensor.matmul(
                        ph[:, :R1],
                        Wp1T[:, k, m * 128 : (m + 1) * 128],
                        xT[:, k, 0:R1],
                        start=(k == 0),
                        stop=(k == KT - 1),
                    )
                nc.vector.tensor_copy(h1T[:, m, :], ph[:, :R1])
                nc.vector.tensor_copy(h1T8[:, m, :], ph[:, :R1])
            h2T = const.tile([64, R2], bf16)
            h2T2 = const.tile([128, 128], bf16)
            ph2 = mainps.tile([128, SC_W], f32, tag="ps", name="ph2")
            for k in range(KT):
                nc.tensor.matmul(
                    ph2[:, :R2],
                    Wp2Td[:, k, :],
                    xT[:, k, R1:R],
                    start=(k == 0),
                    stop=(k == KT - 1),
                )
            nc.vector.tensor_copy(h2T[:, :], ph2[:64, :R2])
            nc.vector.tensor_copy(h2T2[0:64, :], ph2[0:64, 0:128])
            nc.vector.tensor_copy(h2T2[64:128, :], ph2[64:128, 128:256])

            # ---- head phase with t1 infill (ACT slack ~1.6us per head tile) ----
            t1_next = 0
            slab1_cur = None

            def emit_t1(k):
                nonlocal t1_next, slab1_cur
                for _ in range(k):
                    if t1_next >= len(t1_plan) * T1B:
                        return
                    isc, rb = divmod(t1_next, T1B)
                    if rb == 0:
                        slab1_cur = t1_slab(isc)
                    t1_tile(slab1_cur, isc, rb)
                    t1_next += 1

            for isc in range(1, len(head_plan)):
                sH = head_slab(isc)
                for rb in range(G):
                    head_tile(sH, isc, rb)
                    if rb % 2 == 1:
                        emit_t1(1)

            # ---- tail phase: rest of t1 + all t2 ----
            for isc in range(len(t2_plan)):
                emit_t1(2)
                t2_sc(isc)
            emit_t1(10**9)

            WgT = const.tile([128, KT, R], bf16)
            nc.scalar.dma_start(out=WgT, in_=WgT_d[:, :].rearrange("(kt p) r -> p kt r", p=128))
            Wg1T = const.tile([128, 2, R1], bf16)
            nc.scalar.dma_start(out=Wg1T, in_=Wg1T_d[:, :].rearrange("(kt p) r -> p kt r", p=128))
            Wg2T = const.tile([64, R2], bf16)
            nc.scalar.dma_start(out=Wg2T, in_=Wg2T_d[:, :])
            m1 = const.tile([128, T1B], f32)
            nc.scalar.dma_start(out=m1, in_=m1_d[:, :])
            m2 = const.tile([128, T2B], f32)
            nc.scalar.dma_start(out=m2, in_=m2_d[:, :])
            mv = const.tile([128, G], f32)
            nc.scalar.dma_start(out=mv, in_=mv_d[:, :])
            ident = const.tile([128, 128], f32)
            make_identity(nc, ident[:, :])
            ones = const.tile([128, 1], f32)
            nc.vector.memset(ones, 1.0)


            # ---- stage E: target-logit dots (cross-matmul + diag reduce) ----
            Th = const.tile([128, G], f32)
            T1 = const.tile([128, T1B], f32)
            T2 = const.tile([128, T2B], f32)
            dscr = const.tile([128, 128], f32)
            for rb in range(G):
                pd = mainps.tile([128, SC_W], f32, tag="ps")
                sl = slice(rb * 128, (rb + 1) * 128)
                for k in range(KT):
                    nc.tensor.matmul(
                        pd[:, :128], WgT[:, k, sl], xT[:, k, sl],
                        start=(k == 0), stop=(k == KT - 1),
                    )
                nc.vector.tensor_mul(dscr[:, :], pd[:, :128], ident[:, :])
                nc.vector.reduce_sum(Th[:, rb : rb + 1], dscr[:, :], axis=X)
            for rb in range(T1B):
                pd = mainps.tile([128, SC_W], f32, tag="ps")
                sl = slice(rb * 128, (rb + 1) * 128)
                for k in range(2):
                    nc.tensor.matmul(
                        pd[:, :128], Wg1T[:, k, sl], h1T[:, k, sl],
                        start=(k == 0), stop=(k == 1),
                    )
                nc.vector.tensor_mul(dscr[:, :], pd[:, :128], ident[:, :])
                nc.vector.reduce_sum(T1[:, rb : rb + 1], dscr[:, :], axis=X)
            for rb in range(T2B):
                pd = mainps.tile([128, SC_W], f32, tag="ps")
                sl = slice(rb * 128, (rb + 1) * 128)
                nc.tensor.matmul(
                    pd[:, :128], Wg2T[:, sl], h2T[:, sl], start=True, stop=True
                )
                nc.vector.tensor_mul(dscr[:, :], pd[:, :128], ident[:, :])
                nc.vector.reduce_sum(T2[:, rb : rb + 1], dscr[:, :], axis=X)

            # ---- stage F: reduce sums, lse, assemble ----
            S_h = const.tile([128, G], f32)
            nc.vector.reduce_sum(S_h, ssum_h, axis=X)
            S_1 = const.tile([128, T1B], f32)
            nc.vector.reduce_sum(S_1, ssum_1, axis=X)
            S_2 = const.tile([128, T2B], f32)
            nc.vector.reduce_sum(S_2, ssum_2, axis=X)

            lse_h = const.tile([128, G], f32)
            nc.scalar.activation(out=lse_h, in_=S_h, func=AF.Ln)
            lse_1 = const.tile([128, T1B], f32)
            nc.scalar.activation(out=lse_1, in_=S_1, func=AF.Ln)
            lse_2 = const.tile([128, T2B], f32)
            nc.scalar.activation(out=lse_2, in_=S_2, func=AF.Ln)

            og = const.tile([128, G], f32)
            nc.vector.tensor_sub(og, Th, lse_h)
            v1 = const.tile([128, T1B], f32)
            nc.vector.tensor_sub(v1, T1, lse_1)
            nc.vector.tensor_mul(v1, v1, m1)
            nc.vector.tensor_add(og[:, 0:T1B], og[:, 0:T1B], v1)
            v2 = const.tile([128, T2B], f32)
            nc.vector.tensor_sub(v2, T2, lse_2)
            nc.vector.tensor_mul(v2, v2, m2)
            nc.vector.tensor_add(og[:, T1B:G], og[:, T1B:G], v2)

            lsrc = const.tile([128, G], f32)
            nc.vector.tensor_mul(lsrc, og, mv)
            pl = mainps.tile([128, SC_W], f32, tag="ps")
            nc.tensor.matmul(pl[:1, :G], ones[:, :], lsrc[:, :], start=True, stop=True)
            lsum = const.tile([1, 1], f32)
            nc.vector.reduce_sum(lsum, pl[:1, :G], axis=X)

            nc.sync.dma_start(out=og_d[:, :], in_=og)
            nc.sync.dma_start(out=loss_d[:, :], in_=lsum)

    nc.compile()
    _NC_CACHE[key] = nc
    return nc


def kernel(x, target, W_head, Wp1, Wo1, Wp2, Wo2):
    from concourse.bass_utils import run_bass_kernel_spmd

    x = np.asarray(x, dtype=np.float32)
    t = np.asarray(target).astype(np.int64)
    W_head = np.asarray(W_head, dtype=np.float32)
    Wp1 = np.asarray(Wp1, dtype=np.float32)
    Wo1 = np.asarray(Wo1, dtype=np.float32)
    Wp2 = np.asarray(Wp2, dtype=np.float32)
    Wo2 = np.asarray(Wo2, dtype=np.float32)
    n = x.shape[0]

    # ---- host routing (adaptive-softmax dispatch) ----
    gather_inds = np.where(t < C0, t, np.where(t < C1, SHORT, SHORT + 1))
    rel1 = np.clip(t - C0, 0, T1_SIZE - 1)
    rel2 = np.clip(t - C1, 0, T2_SIZE - 1)
    in1 = (t >= C0) & (t < C1)
    in2 = (t >= C1) & (t < C2)
    idx1 = np.nonzero(in1)[0]
    idx2 = np.nonzero(in2)[0]
    idx0 = np.nonzero(~(in1 | in2))[0]

    per1 = [list(idx1[c::NCORES]) for c in range(NCORES)]
    per2 = [list(idx2[c::NCORES]) for c in range(NCORES)]
    fill = list(idx0)
    rpc = n // NCORES
    n1max = max(len(l) for l in per1)
    n2max = max(len(l) for l in per2)
    T1B = max(1, math.ceil(n1max / 128))
    T2B = max(1, math.ceil(n2max / 128))
    while (T1B + T2B) * 128 < rpc:
        if T1B <= T2B:
            T1B += 1
        else:
            T2B += 1
    G = T1B + T2B
    R, R1, R2 = G * 128, T1B * 128, T2B * 128

    # assign filler (shortlist) rows: each core needs rpc - n1c - n2c of them
    slot_rows = []  # per core: array of orig row index per slot, -1 = pad
    fpos = 0
    for c in range(NCORES):
        need = rpc - len(per1[c]) - len(per2[c])
        fillers = fill[fpos : fpos + need]
        fpos += need
        gap1 = R1 - len(per1[c])  # tail1-region filler slots
        f1 = fillers[:gap1]
        f2 = fillers[gap1:]
        rows = np.full(R, -1, dtype=np.int64)
        rows[: len(per1[c])] = per1[c]
        rows[len(per1[c]) : len(per1[c]) + len(f1)] = f1
        rows[R1 : R1 + len(per2[c])] = per2[c]
        rows[R1 + len(per2[c]) : R1 + len(per2[c]) + len(f2)] = f2
        slot_rows.append(rows)
    assert fpos == len(fill)

    # ---- shared (replicated) weight arrays ----
    WhT8 = _fp8(np.ascontiguousarray(W_head.T) * W_SCALE)
    Wp1T = np.ascontiguousarray(Wp1.T).astype(BF16)
    Wo1T8 = _fp8(np.ascontiguousarray(Wo1.T) * W_SCALE)
    Wp2Td = np.ascontiguousarray(np.concatenate([Wp2.T, Wp2.T], axis=1)).astype(BF16)
    Wo2T = np.ascontiguousarray(Wo2.T).astype(BF16)

    in_maps = []
    for c in range(NCORES):
        rows = slot_rows[c]
        valid = rows >= 0
        rv = np.where(valid, rows, 0)

        xT_f = np.where(valid[None, :], x[rv].T, 0.0)
        xT_c = xT_f.astype(BF16)
        xT8_c = _fp8(xT_f)
        WgT_c = np.where(valid[None, :], W_head[gather_inds[rv]].T, 0.0).astype(BF16)
        r1 = rows[:R1]
        v1 = r1 >= 0
        rv1 = np.where(v1, r1, 0)
        Wg1T_c = np.where(v1[None, :], Wo1[rel1[rv1]].T, 0.0).astype(BF16)
        r2 = rows[R1:]
        v2 = r2 >= 0
        rv2 = np.where(v2, r2, 0)
        Wg2T_c = np.where(v2[None, :], Wo2[rel2[rv2]].T, 0.0).astype(BF16)

        def grid(vec):
            return np.ascontiguousarray(vec.reshape(-1, 128).T).astype(np.float32)

        m1_c = grid((v1 & in1[rv1]).astype(np.float32))
        m2_c = grid((v2 & in2[rv2]).astype(np.float32))
        mv_c = grid(valid.astype(np.float32))

        in_maps.append(
            {
                "xT": np.ascontiguousarray(xT_c),
                "xT8": np.ascontiguousarray(xT8_c),
                "WgT": np.ascontiguousarray(WgT_c),
                "Wg1T": np.ascontiguousarray(Wg1T_c),
                "Wg2T": np.ascontiguousarray(Wg2T_c),
                "WhT8": WhT8,
                "Wp1T": Wp1T,
                "Wo1T8": Wo1T8,
                "Wp2Td": Wp2Td,
                "Wo2T": Wo2T,
                "m1": m1_c,
                "m2": m2_c,
                "mv": mv_c,
            }
        )

    nc = _build_nc(T1B, T2B)
    res = run_bass_kernel_spmd(
        nc,
        in_maps,
        core_ids=list(range(NCORES)),
        trace=bool(os.environ.get("AXS_TRACE")),
    )
    global LAST_RESULT
    LAST_RESULT = res

    out = np.zeros(n, dtype=np.float32)
    loss_sum = 0.0
    for c in range(NCORES):
        rows = slot_rows[c]
        valid = rows >= 0
        flat = np.asarray(res.results[c]["out_grid"]).T.reshape(-1)
        out[rows[valid]] = flat[valid]
        loss_sum += float(np.asarray(res.results[c]["loss_part"]).reshape(-1)[0])
    loss = np.float32(-loss_sum / n)
    return out, loss


LAST_RESULT = None
